# revision 2
# baseline (speedup 1.0000x reference)
"""Causal multi-head attention block (QKV proj -> causal softmax attention -> out proj)
for Trainium2, distributed over 8 NeuronCores.

Sharding: 8 cores = 4 batches x 2 head-groups (6 heads each), as in the
baseline, but the device kernel is restructured for PE saturation:

  - All DRAM parameters are laid out partition-major on the host so every
    input DMA moves multi-KB contiguous lines per partition (the baseline's
    256B-packet storm gated the start by ~20us).
  - QKV chunk projections, v-tile projections and output projections are
    emitted as small "filler" chains woven between attention blocks with
    deadlines, so the PE never waits on the scalar engine's exp backlog and
    the scalar engine never starves.
  - The causal mask is applied *additively* on the PE (identity-stationary
    matmul streaming a -50 masked constant into the diagonal block) instead
    of a post-exp multiply on gpsimd.
  - PV accumulators lag the exp stream by two blocks and own a dedicated
    2-buffer PSUM ring, so the PE never waits on a just-issued exp and
    filler chains never contend with live accumulators.
  - Softmax normalization: denominators (a ones-column in v) go through a
    cheap approximate reciprocal on minimal data, then a gpsimd
    partition_broadcast (baseline burned 79us in 1-partition RECIPROCALs +
    DRAM bounces).
  - Output is stored bf16 (host sums the two head-group partials in fp32
    and adds b_proj), halving store and fetch traffic.

Shapes hardcoded for B=4, S=2048, D=768, H=12, DH=64.
"""

import sys

sys.path.insert(0, "/opt/trn_rl_repo")

from contextlib import ExitStack

import numpy as np
import ml_dtypes

import concourse.mybir as mybir
import concourse.tile as tile
from concourse import bacc

B, S, D, H, DH = 4, 2048, 768, 12, 64
NCORES = 8
HG = 6                # heads per core (head-group)
GD = HG * DH          # 384: per-core qkv width
PAIRS = HG // 2       # 3 head-pairs (one pair = one 128-partition tile)
KT = D // 128         # 6 contraction tiles for the projections
QC = S // 512         # 4 query chunks of 512
JT = S // 128         # 16 key tiles of 128
F32 = mybir.dt.float32
BF16 = mybir.dt.bfloat16
BF16_NP = ml_dtypes.bfloat16
Exp = mybir.ActivationFunctionType.Exp
MUL = mybir.AluOpType.mult
ADD = mybir.AluOpType.add


def _build(with_bias=True):
    nc = bacc.Bacc("TRN2")

    # Partition-major host layouts (see _shard_inputs): every DMA below is
    # contiguous per partition.
    xh = nc.declare_dram_parameter("xh", [128, QC * KT * 512], BF16, isOutput=False)
    wq = nc.declare_dram_parameter("wq", [128, PAIRS * KT * 128], BF16, isOutput=False)
    wk = nc.declare_dram_parameter("wk", [128, PAIRS * KT * 128], BF16, isOutput=False)
    wv = nc.declare_dram_parameter("wv", [128, KT * GD], BF16, isOutput=False)
    wp = nc.declare_dram_parameter("wp", [128, PAIRS * D], BF16, isOutput=False)
    msk = nc.declare_dram_parameter("msk", [128, 128], BF16, isOutput=False)
    eye = nc.declare_dram_parameter("eye", [128, 128], BF16, isOutput=False)
    if with_bias:
        bq = nc.declare_dram_parameter("bq", [128, PAIRS], F32, isOutput=False)
        bk = nc.declare_dram_parameter("bk", [128, PAIRS], F32, isOutput=False)
        bv = nc.declare_dram_parameter("bv", [128, PAIRS], F32, isOutput=False)
    out = nc.declare_dram_parameter("out", [S, D], BF16, isOutput=True)

    xh4 = xh.rearrange("p (c k s) -> p c k s", c=QC, k=KT)
    wq4 = wq.rearrange("p (r k m) -> p r k m", r=PAIRS, k=KT)
    wk4 = wk.rearrange("p (r k m) -> p r k m", r=PAIRS, k=KT)
    wv3 = wv.rearrange("p (k m) -> p k m", k=KT)
    wp3 = wp.rearrange("p (r n) -> p r n", r=PAIRS)

    with tile.TileContext(nc) as tc, ExitStack() as ctx:
        const = ctx.enter_context(tc.tile_pool(name="const", bufs=1))
        big = ctx.enter_context(tc.tile_pool(name="big", bufs=1))
        expp = ctx.enter_context(tc.tile_pool(name="expp", bufs=6))
        small = ctx.enter_context(tc.tile_pool(name="small", bufs=4))
        outp = ctx.enter_context(tc.tile_pool(name="outp", bufs=3))
        ps = ctx.enter_context(tc.tile_pool(name="ps", bufs=2, space="PSUM"))

        wq_sb = const.tile([128, PAIRS, KT, 128], BF16)
        wk_sb = const.tile([128, PAIRS, KT, 128], BF16)
        wv_sb = const.tile([128, KT, GD], BF16)
        wp_sb = const.tile([128, PAIRS, D], BF16)
        msk_sb = const.tile([128, 128], BF16)
        eye_sb = const.tile([128, 128], BF16)
        xh_sb = const.tile([128, QC, KT, 512], BF16)

        # DMA order: first QKV chain needs wq/wk pair 0 + xh chunk 0 — those
        # go first, on two queues in parallel.
        nc.scalar.dma_start(wq_sb[:, 0], wq4[:, 0])
        nc.sync.dma_start(xh_sb[:, 0], xh4[:, 0])
        nc.scalar.dma_start(wk_sb[:, 0], wk4[:, 0])
        nc.scalar.dma_start(wv_sb, wv3[:, :])
        nc.scalar.dma_start(msk_sb, msk[:, :])
        nc.scalar.dma_start(eye_sb, eye[:, :])
        for c in range(1, QC):
            nc.sync.dma_start(xh_sb[:, c], xh4[:, c])
        for r in range(1, PAIRS):
            nc.scalar.dma_start(wq_sb[:, r], wq4[:, r])
            nc.scalar.dma_start(wk_sb[:, r], wk4[:, r])
        nc.scalar.dma_start(wp_sb, wp3[:, :])
        if with_bias:
            bq_sb = const.tile([128, PAIRS], F32)
            bk_sb = const.tile([128, PAIRS], F32)
            bv_sb = const.tile([128, PAIRS], F32)
            nc.scalar.dma_start(bq_sb, bq[:, :])
            nc.scalar.dma_start(bk_sb, bk[:, :])
            nc.scalar.dma_start(bv_sb, bv[:, :])

        # ---- persistent activations ----
        qT_sb = big.tile([128, PAIRS, S], BF16)   # [dh-of-pair, pair, s]
        kT_sb = big.tile([128, PAIRS, S], BF16)
        v_sb = big.tile([128, JT, HG, DH + 1], BF16)  # [s_local, s_tile, head, dh+ones]
        outT_sb = big.tile([128, PAIRS, S], BF16)

        nc.vector.memset(v_sb[:, :, :, DH : DH + 1], 1.0)

        # ---------- filler machinery ----------
        # Projection chains are emitted between attention blocks so the PE
        # has independent work while the scalar engine chews the exp
        # backlog. Items carry a PE cost (columns) and a deadline (pair, qc)
        # before which they MUST be emitted (data dependencies of the
        # attention stream itself).
        fillers = []          # list of (deadline, cols, emit_fn), in order
        budget = [0.0]        # accumulated columns available to spend

        def pump(cols=0.0, deadline=None):
            budget[0] += cols
            while fillers:
                dl, cost, fn = fillers[0]
                due = deadline is not None and dl <= deadline
                if not due and budget[0] < cost:
                    break
                fillers.pop(0)
                fn()
                budget[0] -= cost

        def qk_chunk(pr, c):
            """qT and kT for pair pr, s-chunk c."""
            for which in range(2):
                w_sb, dst = (wq_sb, qT_sb) if which == 0 else (wk_sb, kT_sb)
                acc = ps.tile([128, 512], F32, tag="b1", bufs=2, name="qk_ps")
                for kt in range(KT):
                    nc.tensor.matmul(
                        acc,
                        lhsT=w_sb[:, pr, kt],
                        rhs=xh_sb[:, c, kt],
                        start=(kt == 0),
                        stop=(kt == KT - 1),
                    )
                dstv = dst[:, pr, c * 512 : (c + 1) * 512]
                if with_bias:
                    bt = bq_sb if which == 0 else bk_sb
                    nc.vector.tensor_tensor(
                        dstv, acc, bt[:, pr : pr + 1].to_broadcast((128, 512)), ADD
                    )
                else:
                    nc.vector.tensor_copy(out=dstv, in_=acc)

        def proj_v(st):
            """v s-tile st: psum[s_local, hd] = sum_D x[s, D] * wv[D, hd]."""
            c, r = divmod(st, 4)
            acc = ps.tile([128, GD], F32, tag="b1", bufs=2, name="v_ps")
            for kt in range(KT):
                nc.tensor.matmul(
                    acc,
                    lhsT=xh_sb[:, c, kt, r * 128 : (r + 1) * 128],
                    rhs=wv_sb[:, kt],
                    start=(kt == 0),
                    stop=(kt == KT - 1),
                )
            nc.vector.tensor_copy(
                out=v_sb[:, st, :, 0:DH],
                in_=acc.rearrange("p (h d) -> p h d", h=HG),
            )

        def proj_out(qt, dma_eng=None):
            eng = dma_eng if dma_eng is not None else nc.sync
            stage = outp.tile([128, D], BF16, tag="stage", name="stage")
            for nch in range(2):
                acc = ps.tile([128, GD], F32, tag="b1", bufs=2, name="o_ps")
                for kt in range(PAIRS):
                    nc.tensor.matmul(
                        acc,
                        lhsT=outT_sb[:, kt, qt * 128 : (qt + 1) * 128],
                        rhs=wp_sb[:, kt, nch * GD : (nch + 1) * GD],
                        start=(kt == 0),
                        stop=(kt == PAIRS - 1),
                    )
                nc.vector.tensor_copy(out=stage[:, nch * GD : (nch + 1) * GD], in_=acc)
            eng.dma_start(out[qt * 128 : (qt + 1) * 128, :], stage)

        def normalize(p, qc, pv):
            """outT[dh, q] = pv[dh, q] * (1 / pv[64, q])  (+ v bias).

            Ordered to release the pv PSUM banks as early as possible (they
            gate the next chunk's PV accumulators and the filler chains): the
            raw pv and the denominator row are copied out first; the cheap
            approximate reciprocal, gpsimd partition-broadcast and in-place
            scale then run off the PE-critical path."""
            dn = small.tile([1, 2, 512], F32, tag="dn", name="dn")
            for h2 in range(2):
                nc.vector.tensor_copy(out=dn[:, h2, :], in_=pv[h2][DH : DH + 1, :])
            rc = small.tile([1, 2, 512], F32, tag="rc", name="rc")
            nc.vector.reciprocal_approx_fast(rc, dn)
            bc = small.tile([64, 2, 512], F32, tag="bc", name="bc")
            for h2 in range(2):
                nc.gpsimd.partition_broadcast(bc[:, h2, :], rc[:, h2, :])
            for h2 in range(2):
                dst = outT_sb[64 * h2 : 64 * h2 + 64, p, qc * 512 : (qc + 1) * 512]
                nc.vector.tensor_tensor(dst, pv[h2][0:DH, :], bc[:, h2, :], MUL)
                if with_bias:
                    nc.vector.tensor_tensor(
                        dst,
                        dst,
                        bv_sb[64 * h2 : 64 * h2 + 64, p : p + 1].to_broadcast((64, 512)),
                        ADD,
                    )

        def attn_pair(p, qcs, ratio, after_qc=None):
            """Causal attention for head pair p over query chunks `qcs` as one
            flat software pipeline; between blocks, filler chains are pumped
            at `ratio` columns per attention column."""
            pvs = {}
            pend = []  # queue of (qc, jt, e, cs); PV lags exp by 2 blocks

            def flush(item):
                qc, jt, e, cs = item
                njt = 4 * qc + 4
                if qc not in pvs:
                    pvs[qc] = [
                        ps.tile([DH + 1, 512], F32, tag="pv", bufs=2, name=f"pv{h2}")
                        for h2 in range(2)
                    ]
                pv = pvs[qc]
                for h2 in range(2):
                    nc.tensor.matmul(
                        pv[h2][:, cs:512],
                        lhsT=v_sb[:, jt, 2 * p + h2, :],
                        rhs=e[:, h2, cs:512],
                        start=(jt == 0),
                        stop=(jt == njt - 1),
                    )
                if jt == njt - 1:
                    normalize(p, qc, pv)
                    del pvs[qc]
                    if after_qc is not None:
                        after_qc(qc)

            for qc in qcs:
                pump(deadline=(p, qc))  # force-drain items due before this chunk
                for jt in range(4 * qc + 4):
                    t = jt - 4 * qc
                    cs = 128 * t if t >= 0 else 0
                    sc = ps.tile([128, 2, 512], F32, tag="sc", bufs=2, name="sc")
                    for h2 in range(2):
                        nc.tensor.matmul(
                            sc[:, h2, cs:512],
                            lhsT=kT_sb[64 * h2 : 64 * h2 + 64, p, jt * 128 : (jt + 1) * 128],
                            rhs=qT_sb[64 * h2 : 64 * h2 + 64, p, qc * 512 + cs : (qc + 1) * 512],
                            start=True,
                            stop=True,
                        )
                    if t >= 0:
                        # additive causal mask on the diagonal 128x128 block:
                        # psum[cs:cs+128] += (I.T @ msk) = msk  (msk = -50
                        # strictly above the diagonal, 0 elsewhere).
                        # skip_group_check: the scores matmul above already
                        # closed its accumulation group over the full span;
                        # this is a plain per-element accumulate on HW.
                        for h2 in range(2):
                            nc.tensor.matmul(
                                sc[:, h2, cs : cs + 128],
                                lhsT=eye_sb,
                                rhs=msk_sb,
                                start=False,
                                stop=True,
                                skip_group_check=True,
                            )
                    e = expp.tile([128, 2, 512], BF16, tag="e", name="e")
                    nc.scalar.activation(e[:, :, cs:512], sc[:, :, cs:512], Exp)
                    pump(ratio * 4 * (512 - cs))
                    if len(pend) >= 2:
                        flush(pend.pop(0))
                    pend.append((qc, jt, e, cs))
            while pend:
                flush(pend.pop(0))

        # ---------- emission schedule ----------
        qk_chunk(0, 0)
        for st in range(4):
            proj_v(st)

        # Filler queue for pairs 0 and 1: remaining projections, each tagged
        # with the (pair, chunk) before which it must be emitted.
        for c in range(1, QC):
            fillers.append(((0, c), 6144, lambda c=c: qk_chunk(0, c)))
            for st in range(4 * c, 4 * c + 4):
                fillers.append(((0, c), 2304, lambda st=st: proj_v(st)))
        for c in range(QC):
            fillers.append(((1, c), 6144, lambda c=c: qk_chunk(1, c)))
        for c in range(QC):
            # pair 2 runs its chunks in reverse: all its qT/kT chunks are
            # needed at its first chunk (qc=3)
            fillers.append(((2, 3), 6144, lambda c=c: qk_chunk(2, c)))

        attn_pair(0, range(QC), ratio=0.67)
        attn_pair(1, range(QC), ratio=0.72)

        # Pair 2 reversed: each finished chunk releases its 4 q-tiles for
        # the output projection, drained as filler during later chunks.
        def after_p2(qc):
            last = qc == 0
            for qt in range(4 * qc, 4 * qc + 4):
                fillers.append(
                    (
                        (2, 99),
                        2304,
                        lambda qt=qt, last=last: proj_out(
                            qt, dma_eng=nc.scalar if last else None
                        ),
                    )
                )

        attn_pair(2, [3, 2, 1, 0], ratio=0.55, after_qc=after_p2)
        pump(deadline=(99, 99))  # drain remaining proj_out chains

    nc.finalize()
    return nc


_CACHE = {}


def _get_nc(with_bias=True):
    key = ("nc", with_bias)
    if key not in _CACHE:
        _CACHE[key] = _build(with_bias)
    return _CACHE[key]


def _shard_inputs(x, W_attn, b_attn, W_proj, with_bias):
    xhs = [
        np.ascontiguousarray(
            x[b].astype(BF16_NP).reshape(QC, 512, KT, 128).transpose(3, 0, 2, 1)
        ).reshape(128, QC * KT * 512)
        for b in range(B)
    ]
    k = np.arange(128)[:, None]
    j = np.arange(128)[None, :]
    msk = np.where(k > j, -50.0, 0.0).astype(BF16_NP)
    eye = np.eye(128, dtype=BF16_NP)
    gshard = []
    for g in range(2):
        cs = slice(g * GD, (g + 1) * GD)
        wqg = W_attn[:, 0 * D : 1 * D][:, cs].astype(BF16_NP)
        wkg = W_attn[:, 1 * D : 2 * D][:, cs].astype(BF16_NP)
        wvg = W_attn[:, 2 * D : 3 * D][:, cs].astype(BF16_NP)
        wpg = W_proj[cs, :].astype(BF16_NP)
        sh = {
            "wq": np.ascontiguousarray(
                wqg.reshape(KT, 128, PAIRS, 128).transpose(1, 2, 0, 3)
            ).reshape(128, PAIRS * KT * 128),
            "wk": np.ascontiguousarray(
                wkg.reshape(KT, 128, PAIRS, 128).transpose(1, 2, 0, 3)
            ).reshape(128, PAIRS * KT * 128),
            "wv": np.ascontiguousarray(
                wvg.reshape(KT, 128, GD).transpose(1, 0, 2)
            ).reshape(128, KT * GD),
            "wp": np.ascontiguousarray(
                wpg.reshape(PAIRS, 128, D).transpose(1, 0, 2)
            ).reshape(128, PAIRS * D),
            "msk": msk,
            "eye": eye,
        }
        if with_bias:
            sh["bq"] = np.ascontiguousarray(
                b_attn[0 * D : 1 * D][cs].reshape(PAIRS, 128).T
            ).astype(np.float32)
            sh["bk"] = np.ascontiguousarray(
                b_attn[1 * D : 2 * D][cs].reshape(PAIRS, 128).T
            ).astype(np.float32)
            sh["bv"] = np.ascontiguousarray(
                b_attn[2 * D : 3 * D][cs].reshape(PAIRS, 128).T
            ).astype(np.float32)
        gshard.append(sh)
    return [{"xh": xhs[c // 2], **gshard[c % 2]} for c in range(NCORES)]


def _get_runner(with_bias=True):
    """Build (once) a cached jitted shard_map executable over the 8 cores.

    Differs from the baseline runner in one important way: the donated
    output buffers are created on-device inside the jitted body instead of
    being 50MB of host zeros transferred through the relay every call.
    """
    rkey = ("runner", with_bias)
    if rkey in _CACHE:
        return _CACHE[rkey]

    import jax
    import jax.numpy as jnp
    from jax.sharding import Mesh, PartitionSpec
    from jax.experimental.shard_map import shard_map
    from concourse import bass2jax
    from concourse import mybir as mb

    nc = _get_nc(with_bias)
    bass2jax.install_neuronx_cc_hook()

    partition_name = nc.partition_id_tensor.name if nc.partition_id_tensor else None
    in_names, out_names, out_avals = [], [], []
    for alloc in nc.m.functions[0].allocations:
        if not isinstance(alloc, mb.MemoryLocationSet):
            continue
        name = alloc.memorylocations[0].name
        if alloc.kind == "ExternalInput":
            if name != partition_name:
                in_names.append(name)
        elif alloc.kind == "ExternalOutput":
            out_names.append(name)
            shape = tuple(alloc.tensor_shape)
            dtype = mb.dt.np(alloc.dtype)
            out_avals.append(jax.core.ShapedArray(shape, dtype))
    all_names = list(in_names) + out_names
    if partition_name is not None:
        all_names.append(partition_name)
    n_params = len(in_names)

    def _body(*args):
        operands = list(args)
        if partition_name is not None:
            operands.append(bass2jax.partition_id_tensor())
        outs = bass2jax._bass_exec_p.bind(
            *operands,
            out_avals=tuple(out_avals),
            in_names=tuple(all_names),
            out_names=tuple(out_names),
            lowering_input_output_aliases=(),
            sim_require_finite=True,
            sim_require_nnan=True,
            nc=nc,
        )
        return tuple(outs)

    devices = jax.devices()[:NCORES]
    mesh = Mesh(np.asarray(devices), ("core",))
    n_outs = len(out_avals)
    in_specs = (PartitionSpec("core"),) * (n_params + n_outs)
    out_specs = (PartitionSpec("core"),) * n_outs
    donate = tuple(range(n_params, n_params + n_outs))
    sharded = jax.jit(
        shard_map(
            _body, mesh=mesh, in_specs=in_specs, out_specs=out_specs, check_rep=False
        ),
        donate_argnums=donate,
        keep_unused=True,
    )
    # Donated output buffers are created on-device (sharded zeros) instead of
    # shipping 8x zero arrays through the host->device relay every call.
    from jax.sharding import NamedSharding

    zsharding = NamedSharding(mesh, PartitionSpec("core"))
    zeros_fn = jax.jit(
        lambda: tuple(
            jnp.zeros((NCORES * a.shape[0], *a.shape[1:]), a.dtype) for a in out_avals
        ),
        out_shardings=(zsharding,) * n_outs,
    )

    def run(in_maps):
        concat_in = [
            np.concatenate([in_maps[c][name] for c in range(NCORES)], axis=0)
            for name in in_names
        ]
        out_arrs = sharded(*concat_in, *zeros_fn())
        return [
            {
                name: np.asarray(out_arrs[i]).reshape(NCORES, *out_avals[i].shape)[c]
                for i, name in enumerate(out_names)
            }
            for c in range(NCORES)
        ]

    _CACHE[rkey] = run
    return run


def _run(x, W_attn, b_attn, W_proj, b_proj):
    x = np.asarray(x, dtype=np.float32)
    W_attn = np.asarray(W_attn, dtype=np.float32)
    b_attn = np.asarray(b_attn, dtype=np.float32)
    W_proj = np.asarray(W_proj, dtype=np.float32)
    b_proj = np.asarray(b_proj, dtype=np.float32)

    with_bias = bool(np.any(b_attn))
    in_maps = _shard_inputs(x, W_attn, b_attn, W_proj, with_bias)
    results = _get_runner(with_bias)(in_maps)
    full = np.empty((B, S, D), dtype=np.float32)
    for b in range(B):
        full[b] = (
            results[2 * b]["out"].astype(np.float32)
            + results[2 * b + 1]["out"].astype(np.float32)
            + b_proj
        )
    return full, results


def kernel(x, W_attn, b_attn, W_proj, b_proj):
    full, _ = _run(x, W_attn, b_attn, W_proj, b_proj)
    return full
